# revision 33
# baseline (speedup 1.0000x reference)
"""Trainium2 Bass kernel for nn_MessageAggregationAttention.

Shards B=256 graphs across 8 NeuronCores (32 graph-slots each). The host does
all data layout (gather of incoming-message rows, per-slot padding,
feature-major transposes) and weight-only algebra:

  - graphs are sorted by key-tile count then query count and dealt
    round-robin to the 8 cores, so slot s has the same static padded sizes
    (lq[s], key tile rows) on every core; the program is compiled for that
    size signature (ragged per-slot matmul shapes),
  - logits are computed as x_q^T (s Wq_h^T Wk_h) x_k: the per-head weight
    product Wqk_h is precomputed on the host, so the device needs NO separate
    K projection; the q-side bias s Wk_h^T bq_h folds into the q bias, the
    k-side bias only shifts whole softmax columns and drops exactly,
  - per (slot, key-tile): logits matmul (lhsT = raw xkT) -> Exp (key-padding
    mask as activation bias) -> two "pair" matmuls with an ones-augmented V
    (lhsT = [ones64 | v_h0 v_h1]) accumulating ctx for two heads AND the
    softmax denominator (partitions 0:64) in one pass,
  - normalize: reciprocal + one pair-flatten copy (vector) + SBUF-only
    per-head multiplies (gpsimd; gpsimd cannot touch PSUM),
  - batched out-projection and FFN with fused bias+residual adds.
"""

import math

import ml_dtypes
import numpy as np

import concourse.mybir as mybir
from concourse import bacc
from concourse.bass_utils import run_bass_kernel_spmd
from concourse.tile import TileContext

B, E, M, H, NH = 256, 16384, 65536, 128, 4
HD = H // NH               # 32
NCORES = 8
G = B // NCORES            # 32 graph slots per core
LQMAX, LKMAX = 96, 320
MASK_VAL = -100.0

f32 = mybir.dt.float32
bf16 = mybir.dt.bfloat16

AFT = mybir.ActivationFunctionType
ALU = mybir.AluOpType

LAST_RESULTS = None
TRACE = False
TRACE_KW = {}


def _build_program(lq, rows_s):
    """lq[s]: padded query count of slot s; rows_s[s]: key-tile row counts
    (e.g. [128, 128, 48])."""
    nc = bacc.Bacc("TRN2")

    qoff = np.r_[0, np.cumsum(lq)]
    QS = int(qoff[-1])
    NQB = -(-QS // 512)
    QSP = NQB * 512
    koff_t = []                        # (slot, tile) -> key-slab col offset
    ko = 0
    for s in range(G):
        offs = []
        for r in rows_s[s]:
            offs.append(ko)
            ko += r
        koff_t.append(offs)
    KS = ko
    toff = np.r_[0, np.cumsum([len(r) for r in rows_s])]  # mask col offsets
    NT = int(toff[-1])

    xqbf_d = nc.dram_tensor("xqbf", [H, QSP], bf16, kind="ExternalInput")
    xkT_d = nc.dram_tensor("xkT", [H, KS], bf16, kind="ExternalInput")
    maskk_d = nc.dram_tensor("maskk", [128, NT], f32, kind="ExternalInput")
    wqk_d = nc.dram_tensor("wqk", [H, 4 * H], bf16, kind="ExternalInput")
    bqz_d = nc.dram_tensor("bqz", [H, 4], f32, kind="ExternalInput")
    wvT_d = nc.dram_tensor("wvT", [H, H], bf16, kind="ExternalInput")
    woT_d = nc.dram_tensor("woT", [H, H], bf16, kind="ExternalInput")
    w1T_d = nc.dram_tensor("w1T", [H, 2 * H], bf16, kind="ExternalInput")
    w2Ta_d = nc.dram_tensor("w2Ta", [128, H], bf16, kind="ExternalInput")
    w2Tb_d = nc.dram_tensor("w2Tb", [128, H], bf16, kind="ExternalInput")
    boc_d = nc.dram_tensor("boc", [H, 1], f32, kind="ExternalInput")
    b1c_d = nc.dram_tensor("b1c", [H, 2], f32, kind="ExternalInput")
    b2c_d = nc.dram_tensor("b2c", [H, 1], f32, kind="ExternalInput")

    out_d = nc.dram_tensor("out", [H, QSP], bf16, kind="ExternalOutput")

    NVR = 4
    with TileContext(nc) as tc:
        with (
            tc.tile_pool(name="const", bufs=1) as constp,
            tc.tile_pool(name="exp", bufs=12) as expp,
            tc.tile_pool(name="rden", bufs=3) as rdenp,
            tc.tile_pool(name="craw", bufs=4) as crawp,
            tc.tile_pool(name="ffn", bufs=3) as ffnp,
            tc.tile_pool(name="ps_big", bufs=2, space="PSUM") as ps_bigp,
            tc.tile_pool(name="ps_lg", bufs=3, space="PSUM") as ps_lgp,
            tc.tile_pool(name="ps_pair", bufs=3, space="PSUM") as ps_pairp,
        ):
            dma_engs = [nc.sync, nc.gpsimd, nc.scalar]
            dma_rr = [0]

            def _dma(out, in_):
                # parallel DGE issuance only for the first (startup-critical)
                # loads; everything later stays on the idle sync queue
                e = dma_engs[dma_rr[0] % 3] if dma_rr[0] < 6 else nc.sync
                e.dma_start(out=out, in_=in_)
                dma_rr[0] += 1

            def _load(shape, dram, dt=f32, nsplit=1):
                t = constp.tile(shape, dt, tag=dram.name, name=dram.name + "_sb")
                w = shape[1]
                step = -(-w // nsplit)
                for i in range(0, w, step):
                    j = min(i + step, w)
                    _dma(t[:, i:j], dram[:, i:j])
                return t

            # DMA issue order matters for the startup critical path:
            # V-proj (wvT + xkT c0) starts first, then the Q side.
            wqk = _load([H, 4 * H], wqk_d, bf16, nsplit=3)
            bqz = _load([H, 4], bqz_d)
            # xqbf blocks gate qproj; 4-slot blocks also gate logits
            QBN = G // 4
            qb_of = [s // 4 for s in range(G)]
            qb_lo = [int(qoff[4 * b]) for b in range(QBN)]
            qb_hi = [int(qoff[4 * b + 4]) for b in range(QBN)]
            xqbf_b = []
            for b in range(QBN):
                w = qb_hi[b] - qb_lo[b]
                t = constp.tile([128, w], bf16, tag=f"xqbf{b}", name=f"xqbf{b}")
                _dma(t[:], xqbf_d[:, qb_lo[b] : qb_hi[b]])
                xqbf_b.append(t)
            wvT = _load([H, H], wvT_d, bf16)
            maskk = _load([128, NT], maskk_d)
            # xkT chunks of 8 slots
            NCH = 4
            ch_of = [s // 8 for s in range(G)]
            ch_lo = [koff_t[8 * c][0] for c in range(NCH)]
            ch_hi = [koff_t[8 * c + 7][-1] + rows_s[8 * c + 7][-1]
                     for c in range(NCH)]
            xkT_c = []
            for c in range(NCH):
                w = ch_hi[c] - ch_lo[c]
                t = constp.tile([128, w], bf16, tag=f"xkT{c}", name=f"xkT{c}")
                step = -(-w // 2)
                for n, i in enumerate(range(0, w, step)):
                    j = min(i + step, w)
                    _dma(t[:, i:j], xkT_d[:, ch_lo[c] + i : ch_lo[c] + j])
                xkT_c.append(t)
            xqT_b = []
            for b5 in range(NQB):
                t = constp.tile([128, 512], bf16, tag=f"xqTb{b5}",
                                name=f"xqTb{b5}")
                _dma(t[:], xqbf_d[:, b5 * 512 : (b5 + 1) * 512])
                xqT_b.append(t)
            woT = _load([H, H], woT_d, bf16)
            w1T = _load([H, 2 * H], w1T_d, bf16)
            w2Ta = _load([128, H], w2Ta_d, bf16)
            w2Tb = _load([128, H], w2Tb_d, bf16)
            boc = _load([H, 1], boc_d)
            b1c = _load([H, 2], b1c_d)
            b2c = _load([H, 1], b2c_d)

            qTz_b = [constp.tile([128, 4, qb_hi[b] - qb_lo[b]], bf16,
                                 tag=f"qTz{b}", name=f"qTz{b}")
                     for b in range(QBN)]
            ctxn = constp.tile([128, QSP], bf16, tag="ctxn", name="ctxn")
            ar = constp.tile([128, QSP], f32, tag="ar", name="ar")
            arbf = constp.tile([128, QSP], bf16, tag="arbf", name="arbf")
            fin = constp.tile([128, QSP], bf16, tag="fin", name="fin")
            if QSP > QS:
                nc.vector.memset(ctxn[:, QS:QSP], 0.0)

            # v_aug ring: per slot [128, T, 256]:
            # [ones64 | v_h0 v_h1 | ones64 | v_h2 v_h3]
            varing = []
            for j in range(NVR):
                va = constp.tile([128, 3, 256], bf16, tag=f"vaug{j}",
                                 name=f"vaug{j}")
                for t in range(3):
                    nc.vector.memset(va[:, t, 0:64], 1.0)
                    nc.vector.memset(va[:, t, 128:192], 1.0)
                varing.append(va)

            PRE = 0

            def emit_v(g):
                c = ch_of[g]
                T = len(rows_s[g])
                psvg = ps_lgp.tile([128, 384], f32, tag="lg")
                for t in range(T):
                    r = rows_s[g][t]
                    off = koff_t[g][t] - ch_lo[c]
                    nc.tensor.matmul(
                        out=psvg[0:r, t * 128 : (t + 1) * 128],
                        lhsT=xkT_c[c][:, off : off + r],
                        rhs=wvT[:], start=True, stop=True,
                        skip_group_check=True,
                    )
                va = varing[g % NVR]
                nc.vector.tensor_copy(
                    out=va[:, 0:T].rearrange(
                        "p t (s f) -> p t s f", s=2)[:, :, :, 64:128],
                    in_=psvg[:, 0 : T * 128].rearrange(
                        "p (t s f) -> p t s f", t=T, s=2),
                )

            # ---- Q projection with folded Wqk ----
            for b in range(QBN):
                for h in range(4):
                    w = qb_hi[b] - qb_lo[b]
                    ps = ps_bigp.tile([128, 512], f32, tag="big")
                    nc.tensor.matmul(
                        out=ps[:, 0:w], lhsT=wqk[:, h * 128 : (h + 1) * 128],
                        rhs=xqbf_b[b][:], start=True, stop=True,
                    )
                    if h % 2 == 0:
                        nc.vector.tensor_scalar_add(
                            out=qTz_b[b][:, h, :], in0=ps[:, 0:w],
                            scalar1=bqz[:, h : h + 1],
                        )
                    else:
                        nc.scalar.activation(
                            out=qTz_b[b][:, h, :], in_=ps[:, 0:w],
                            func=AFT.Identity, bias=bqz[:, h : h + 1],
                        )

            # ---- per-slot attention, software-pipelined ----
            ex_g = {}
            pend = {}

            def emit_logits(g):
                c = ch_of[g]
                L = int(lq[g])
                b = qb_of[g]
                qo = int(qoff[g]) - qb_lo[b]
                exs = []
                for t in range(len(rows_s[g])):
                    r = rows_s[g][t]
                    off = koff_t[g][t] - ch_lo[c]
                    lg_ps = ps_lgp.tile([128, 384], f32, tag="lg")
                    nc.tensor.matmul(
                        out=lg_ps[0:r, 0 : 4 * L],
                        lhsT=xkT_c[c][:, off : off + r],
                        rhs=qTz_b[b][:, :, qo : qo + L],
                        start=True, stop=True,
                    )
                    ex = expp.tile([128, 4 * LQMAX], bf16, tag="ex")
                    kt = int(toff[g]) + t
                    nc.scalar.activation(
                        out=ex[0:r, 0 : 4 * L], in_=lg_ps[0:r, 0 : 4 * L],
                        func=AFT.Exp, bias=maskk[0:r, kt : kt + 1],
                    )
                    exs.append(ex)
                ex_g[g] = exs

            def emit_pairs(g):
                exs = ex_g.pop(g)
                L = int(lq[g])
                va = varing[g % NVR]
                pair = ps_pairp.tile([128, 4 * LQMAX], f32, tag="pair")
                T = len(rows_s[g])
                # two accumulation groups in one PSUM bank must NOT interleave
                for half in range(2):
                    for t in range(T):
                        r = rows_s[g][t]
                        nc.tensor.matmul(
                            out=pair[0:128, 2 * L * half : 2 * L * (half + 1)],
                            lhsT=va[0:r, t, 128 * half : 128 * (half + 1)],
                            rhs=exs[t][0:r, 2 * L * half : 2 * L * (half + 1)],
                            start=(t == 0), stop=(t == T - 1),
                            skip_group_check=True,
                        )
                rdb = rdenp.tile([64, 4 * LQMAX], f32, tag="rdb")
                nc.vector.reciprocal_approx_fast(
                    out=rdb[:, 0 : 4 * L], in_=pair[0:64, 0 : 4 * L])
                craw = crawp.tile([64, 4 * LQMAX], bf16, tag="craw")
                nc.vector.tensor_copy(
                    out=craw[:, 0 : 4 * L], in_=pair[64:128, 0 : 4 * L])
                qc = int(qoff[g])
                for h in range(4):
                    ro = 32 * (h % 2)
                    cs = slice(h * L, (h + 1) * L)
                    nc.gpsimd.tensor_mul(
                        out=ctxn[32 * h : 32 * (h + 1), qc : qc + L],
                        in0=craw[ro : ro + 32, cs],
                        in1=rdb[ro : ro + 32, cs],
                    )

            def emit_outproj(blk):
                sl = slice(blk * 512, (blk + 1) * 512)
                po = ps_bigp.tile([128, 512], f32, tag="big")
                nc.tensor.matmul(
                    out=po[:], lhsT=woT[:], rhs=ctxn[:, sl], start=True, stop=True
                )
                nc.vector.scalar_tensor_tensor(
                    out=ar[:, sl], in0=po[:], scalar=boc[:, 0:1],
                    in1=xqT_b[blk][:], op0=ALU.add, op1=ALU.add,
                )
                nc.vector.tensor_copy(out=arbf[:, sl], in_=ar[:, sl])

            def emit_ffn(blk):
                sl = slice(blk * 512, (blk + 1) * 512)
                pa = ps_bigp.tile([128, 512], f32, tag="big")
                nc.tensor.matmul(
                    out=pa[:], lhsT=w1T[:, 0:128], rhs=arbf[:, sl],
                    start=True, stop=True,
                )
                ra = ffnp.tile([128, 512], bf16, tag="ra")
                nc.scalar.activation(
                    out=ra[:], in_=pa[:], func=AFT.Relu, bias=b1c[:, 0:1]
                )
                pb = ps_bigp.tile([128, 512], f32, tag="big")
                nc.tensor.matmul(
                    out=pb[:], lhsT=w1T[:, 128:256], rhs=arbf[:, sl],
                    start=True, stop=True,
                )
                rb = ffnp.tile([128, 512], bf16, tag="rb")
                nc.scalar.activation(
                    out=rb[:], in_=pb[:], func=AFT.Relu, bias=b1c[:, 1:2]
                )
                p2 = ps_bigp.tile([128, 512], f32, tag="big")
                nc.tensor.matmul(
                    out=p2[:], lhsT=w2Ta[:], rhs=ra[:], start=True, stop=False,
                    skip_group_check=True,
                )
                nc.tensor.matmul(
                    out=p2[:], lhsT=w2Tb[:], rhs=rb[:], start=False, stop=True,
                    skip_group_check=True,
                )
                nc.vector.scalar_tensor_tensor(
                    out=fin[:, sl], in0=p2[:], scalar=b2c[:, 0:1],
                    in1=ar[:, sl], op0=ALU.add, op1=ALU.add,
                )
                if blk >= NQB - 2:
                    for n, i in enumerate(range(sl.start, sl.stop, 128)):
                        e = [nc.sync, nc.gpsimd, nc.scalar][n % 3]
                        e.dma_start(out=out_d[:, i : i + 128],
                                    in_=fin[:, i : i + 128])
                else:
                    nc.sync.dma_start(out=out_d[:, sl.start : sl.start + 256],
                                      in_=fin[:, sl.start : sl.start + 256])
                    nc.sync.dma_start(out=out_d[:, sl.start + 256 : sl.stop],
                                      in_=fin[:, sl.start + 256 : sl.stop])

            # outproj block blk is ready after muls of the slot covering its
            # last column
            blk_after = {}
            for blk in range(NQB):
                end = min(512 * (blk + 1), QS)
                g_ready = min(int(np.searchsorted(qoff[1:], end)), G - 1)
                blk_after.setdefault(g_ready, []).append(blk)

            LAG = 2
            for g in range(G + LAG):
                if g < G:
                    emit_v(g)
                    emit_logits(g)
                if g >= LAG:
                    emit_pairs(g - LAG)
                    for blk in blk_after.get(g - LAG, []):
                        emit_outproj(blk)
                        emit_ffn(blk)
    nc.finalize()
    return nc


_NC_CACHE = {}


def kernel(edge_index, edge_attr, incoming_edges_list, incoming_edges_batch,
           edge_batch, in_proj_w, in_proj_b, out_proj_w, out_proj_b,
           w1, b1, w2, b2):
    global LAST_RESULTS

    edge_attr = np.asarray(edge_attr, np.float32)
    edge_batch = np.asarray(edge_batch, np.int64)
    incoming_edges_list = np.asarray(incoming_edges_list, np.int64)
    incoming_edges_batch = np.asarray(incoming_edges_batch, np.int64)
    bft = ml_dtypes.bfloat16

    cnt_q = np.bincount(edge_batch, minlength=B)
    st_q = np.zeros(B + 1, np.int64)
    np.cumsum(cnt_q, out=st_q[1:])
    cnt_k = np.bincount(incoming_edges_batch, minlength=B)
    st_k = np.zeros(B + 1, np.int64)
    np.cumsum(cnt_k, out=st_k[1:])
    assert cnt_q.max() <= LQMAX and cnt_k.max() <= LKMAX and cnt_k.min() >= 1

    # sort graphs: key-tile count desc, then query count desc; deal
    # round-robin so slot s has similar sizes on all 8 cores
    tiles_g = -(-cnt_k // 128)
    order = np.lexsort((-cnt_q, -tiles_g))          # [B]
    slot_graph = order.reshape(G, NCORES)           # slot s, core c
    qmax_s = cnt_q[slot_graph].max(1)
    kmax_s = cnt_k[slot_graph].max(1)
    lq = np.minimum(-(-qmax_s // 8) * 8, LQMAX).astype(np.int64)
    rows_s = []
    for sI in range(G):
        T = int(-(-kmax_s[sI] // 128))
        last = int(kmax_s[sI]) - 128 * (T - 1)
        rows_s.append([128] * (T - 1) + [min(-(-last // 16) * 16, 128)])
    lk_s = np.array([sum(r) for r in rows_s], np.int64)

    qoff = np.r_[0, np.cumsum(lq)]
    QS = int(qoff[-1])
    NQB = -(-QS // 512)
    QSP = NQB * 512
    koff = np.r_[0, np.cumsum(lk_s)]
    KS = int(koff[-1])
    toff = np.r_[0, np.cumsum([len(r) for r in rows_s])]
    NT = int(toff[-1])

    xz = np.zeros((E + LQMAX, H), np.float32)
    xz[:E] = edge_attr
    xze = np.zeros((E + 1, H), np.float32)
    xze[:E] = edge_attr

    s = 1.0 / math.sqrt(HD)
    wq, wk, wv = in_proj_w[:H], in_proj_w[H:2 * H], in_proj_w[2 * H:]
    bq, bv = in_proj_b[:H], in_proj_b[2 * H:]
    # logits = x_q^T (s Wq_h^T Wk_h) x_k + (s Wk_h^T bq_h) . x_k  (+ col-
    # constant terms that are softmax-invariant and dropped)
    wqk = np.zeros((H, 4 * H), np.float64)
    bqz = np.zeros((H, 4), np.float64)
    for h in range(4):
        hd = slice(32 * h, 32 * (h + 1))
        wqk[:, h * H : (h + 1) * H] = s * (wq[hd].astype(np.float64).T
                                           @ wk[hd].astype(np.float64))
        bqz[:, h] = s * (wk[hd].astype(np.float64).T
                         @ bq[hd].astype(np.float64))

    shared = dict(
        wqk=wqk.astype(bft),
        bqz=bqz.astype(np.float32),
        wvT=np.ascontiguousarray(wv.T.astype(bft)),
        woT=np.ascontiguousarray(out_proj_w.T.astype(bft)),
        w1T=np.ascontiguousarray(w1.T.astype(bft)),
        w2Ta=np.ascontiguousarray(w2.T[0:128].astype(bft)),
        w2Tb=np.ascontiguousarray(w2.T[128:256].astype(bft)),
        boc=np.ascontiguousarray(
            (out_proj_b + out_proj_w @ bv)[:, None], np.float32),
        b1c=np.ascontiguousarray(b1.reshape(2, H).T, np.float32),
        b2c=np.ascontiguousarray(b2[:, None], np.float32),
    )

    in_maps = []
    for c in range(NCORES):
        gs = slot_graph[:, c]
        xq_c = np.zeros((QSP, H), np.float32)
        xk_c = np.zeros((KS, H), np.float32)
        mk = np.full((128, NT), MASK_VAL, np.float32)
        for sI in range(G):
            g = gs[sI]
            L = int(lq[sI])
            xq_c[qoff[sI] : qoff[sI] + L] = xz[st_q[g] : st_q[g] + L]
            n_k = int(cnt_k[g])
            idx = incoming_edges_list[st_k[g] : st_k[g] + n_k]
            xk_c[koff[sI] : koff[sI] + n_k] = xze[idx]
            pos = 0
            for t, r in enumerate(rows_s[sI]):
                v = np.arange(r) + pos
                mk[:r, toff[sI] + t] = np.where(v < n_k, 0.0, MASK_VAL)
                pos += r
        in_maps.append(dict(
            shared,
            xqbf=np.ascontiguousarray(xq_c.T.astype(bft)),
            xkT=np.ascontiguousarray(xk_c.T.astype(bft)),
            maskk=np.ascontiguousarray(mk),
        ))

    key = (tuple(int(x) for x in lq), tuple(tuple(r) for r in rows_s))
    if key not in _NC_CACHE:
        _NC_CACHE.clear()
        _NC_CACHE[key] = _build_program(lq, rows_s)
    res = run_bass_kernel_spmd(
        _NC_CACHE[key], in_maps, core_ids=list(range(NCORES)),
        trace=TRACE, **TRACE_KW,
    )
    LAST_RESULTS = res

    # scatter dense output back to edge rows
    slot_of = np.empty(B, np.int64)
    core_of = np.empty(B, np.int64)
    for sI in range(G):
        for c in range(NCORES):
            slot_of[slot_graph[sI, c]] = sI
            core_of[slot_graph[sI, c]] = c
    eb = edge_batch
    pos = np.arange(E) - st_q[eb]
    col = qoff[slot_of[eb]] + pos
    out_full = np.empty((E, H), np.float32)
    for c in range(NCORES):
        sel = core_of[eb] == c
        out_full[sel] = res.results[c]["out"][:, col[sel]].T.astype(np.float32)
    return out_full


# revision 34
# speedup vs baseline: 1.0227x; 1.0227x over previous
"""Trainium2 Bass kernel for nn_MessageAggregationAttention.

Shards B=256 graphs across 8 NeuronCores (32 graph-slots each). The host does
all data layout (gather of incoming-message rows, per-slot padding,
feature-major transposes) and weight-only algebra:

  - graphs are sorted by key-tile count then query count and dealt
    round-robin to the 8 cores, so slot s has the same static padded sizes
    (lq[s], key tile rows) on every core; the program is compiled for that
    size signature (ragged per-slot matmul shapes),
  - logits are computed as x_q^T (s Wq_h^T Wk_h) x_k: the per-head weight
    product Wqk_h is precomputed on the host, so the device needs NO separate
    K projection; the q-side bias s Wk_h^T bq_h folds into the q bias, the
    k-side bias only shifts whole softmax columns and drops exactly,
  - per (slot, key-tile): logits matmul (lhsT = raw xkT) -> Exp (key-padding
    mask as activation bias) -> two "pair" matmuls with an ones-augmented V
    (lhsT = [ones64 | v_h0 v_h1]) accumulating ctx for two heads AND the
    softmax denominator (partitions 0:64) in one pass,
  - normalize: reciprocal + one pair-flatten copy (vector) + SBUF-only
    per-head multiplies (gpsimd; gpsimd cannot touch PSUM),
  - batched out-projection and FFN with fused bias+residual adds.
"""

import math

import ml_dtypes
import numpy as np

import concourse.mybir as mybir
from concourse import bacc
from concourse.bass_utils import run_bass_kernel_spmd
from concourse.tile import TileContext

B, E, M, H, NH = 256, 16384, 65536, 128, 4
HD = H // NH               # 32
NCORES = 8
G = B // NCORES            # 32 graph slots per core
LQMAX, LKMAX = 96, 320
MASK_VAL = -100.0

f32 = mybir.dt.float32
bf16 = mybir.dt.bfloat16

AFT = mybir.ActivationFunctionType
ALU = mybir.AluOpType

LAST_RESULTS = None
TRACE = False
TRACE_KW = {}


def _build_program(lq, rows_s):
    """lq[s]: padded query count of slot s; rows_s[s]: key-tile row counts
    (e.g. [128, 128, 48])."""
    nc = bacc.Bacc("TRN2")

    qoff = np.r_[0, np.cumsum(lq)]
    QS = int(qoff[-1])
    NQB = -(-QS // 512)
    QSP = NQB * 512
    koff_t = []                        # (slot, tile) -> key-slab col offset
    ko = 0
    for s in range(G):
        offs = []
        for r in rows_s[s]:
            offs.append(ko)
            ko += r
        koff_t.append(offs)
    KS = ko
    toff = np.r_[0, np.cumsum([len(r) for r in rows_s])]  # mask col offsets
    NT = int(toff[-1])

    xqbf_d = nc.dram_tensor("xqbf", [H, QSP], bf16, kind="ExternalInput")
    xkT_d = nc.dram_tensor("xkT", [H, KS], bf16, kind="ExternalInput")
    maskk_d = nc.dram_tensor("maskk", [128, NT], f32, kind="ExternalInput")
    wqk_d = nc.dram_tensor("wqk", [H, 4 * H], bf16, kind="ExternalInput")
    bqz_d = nc.dram_tensor("bqz", [H, 4], f32, kind="ExternalInput")
    wvT_d = nc.dram_tensor("wvT", [H, H], bf16, kind="ExternalInput")
    woT_d = nc.dram_tensor("woT", [H, H], bf16, kind="ExternalInput")
    w1T_d = nc.dram_tensor("w1T", [H, 2 * H], bf16, kind="ExternalInput")
    w2Ta_d = nc.dram_tensor("w2Ta", [128, H], bf16, kind="ExternalInput")
    w2Tb_d = nc.dram_tensor("w2Tb", [128, H], bf16, kind="ExternalInput")
    boc_d = nc.dram_tensor("boc", [H, 1], f32, kind="ExternalInput")
    b1c_d = nc.dram_tensor("b1c", [H, 2], f32, kind="ExternalInput")
    b2c_d = nc.dram_tensor("b2c", [H, 1], f32, kind="ExternalInput")

    out_d = nc.dram_tensor("out", [H, QSP], bf16, kind="ExternalOutput")

    NVR = 4
    with TileContext(nc) as tc:
        with (
            tc.tile_pool(name="const", bufs=1) as constp,
            tc.tile_pool(name="exp", bufs=12) as expp,
            tc.tile_pool(name="rden", bufs=3) as rdenp,
            tc.tile_pool(name="craw", bufs=4) as crawp,
            tc.tile_pool(name="ffn", bufs=3) as ffnp,
            tc.tile_pool(name="ps_big", bufs=2, space="PSUM") as ps_bigp,
            tc.tile_pool(name="ps_lg", bufs=4, space="PSUM") as ps_lgp,
            tc.tile_pool(name="ps_pair", bufs=2, space="PSUM") as ps_pairp,
        ):
            dma_engs = [nc.sync, nc.gpsimd, nc.scalar]
            dma_rr = [0]

            def _dma(out, in_):
                # parallel DGE issuance only for the first (startup-critical)
                # loads; everything later stays on the idle sync queue
                e = dma_engs[dma_rr[0] % 3] if dma_rr[0] < 6 else nc.sync
                e.dma_start(out=out, in_=in_)
                dma_rr[0] += 1

            def _load(shape, dram, dt=f32, nsplit=1):
                t = constp.tile(shape, dt, tag=dram.name, name=dram.name + "_sb")
                w = shape[1]
                step = -(-w // nsplit)
                for i in range(0, w, step):
                    j = min(i + step, w)
                    _dma(t[:, i:j], dram[:, i:j])
                return t

            # DMA issue order matters for the startup critical path:
            # V-proj (wvT + xkT c0) starts first, then the Q side.
            wqk = _load([H, 4 * H], wqk_d, bf16, nsplit=3)
            bqz = _load([H, 4], bqz_d)
            # xqbf blocks gate qproj; 4-slot blocks also gate logits
            QBN = G // 4
            qb_of = [s // 4 for s in range(G)]
            qb_lo = [int(qoff[4 * b]) for b in range(QBN)]
            qb_hi = [int(qoff[4 * b + 4]) for b in range(QBN)]
            xqbf_b = []
            for b in range(QBN):
                w = qb_hi[b] - qb_lo[b]
                t = constp.tile([128, w], bf16, tag=f"xqbf{b}", name=f"xqbf{b}")
                _dma(t[:], xqbf_d[:, qb_lo[b] : qb_hi[b]])
                xqbf_b.append(t)
            wvT = _load([H, H], wvT_d, bf16)
            maskk = _load([128, NT], maskk_d)
            # xkT chunks of 8 slots
            NCH = 4
            ch_of = [s // 8 for s in range(G)]
            ch_lo = [koff_t[8 * c][0] for c in range(NCH)]
            ch_hi = [koff_t[8 * c + 7][-1] + rows_s[8 * c + 7][-1]
                     for c in range(NCH)]
            xkT_c = []
            for c in range(NCH):
                w = ch_hi[c] - ch_lo[c]
                t = constp.tile([128, w], bf16, tag=f"xkT{c}", name=f"xkT{c}")
                step = -(-w // 2)
                for n, i in enumerate(range(0, w, step)):
                    j = min(i + step, w)
                    _dma(t[:, i:j], xkT_d[:, ch_lo[c] + i : ch_lo[c] + j])
                xkT_c.append(t)
            xqT_b = []
            for b5 in range(NQB):
                t = constp.tile([128, 512], bf16, tag=f"xqTb{b5}",
                                name=f"xqTb{b5}")
                _dma(t[:], xqbf_d[:, b5 * 512 : (b5 + 1) * 512])
                xqT_b.append(t)
            woT = _load([H, H], woT_d, bf16)
            w1T = _load([H, 2 * H], w1T_d, bf16)
            w2Ta = _load([128, H], w2Ta_d, bf16)
            w2Tb = _load([128, H], w2Tb_d, bf16)
            boc = _load([H, 1], boc_d)
            b1c = _load([H, 2], b1c_d)
            b2c = _load([H, 1], b2c_d)

            qTz_b = [constp.tile([128, 4, qb_hi[b] - qb_lo[b]], bf16,
                                 tag=f"qTz{b}", name=f"qTz{b}")
                     for b in range(QBN)]
            ctxn = constp.tile([128, QSP], bf16, tag="ctxn", name="ctxn")
            ar = constp.tile([128, QSP], f32, tag="ar", name="ar")
            arbf = constp.tile([128, QSP], bf16, tag="arbf", name="arbf")
            fin = constp.tile([128, QSP], bf16, tag="fin", name="fin")
            if QSP > QS:
                nc.vector.memset(ctxn[:, QS:QSP], 0.0)

            # v_aug ring: per slot [128, T, 256]:
            # [ones64 | v_h0 v_h1 | ones64 | v_h2 v_h3]
            varing = []
            for j in range(NVR):
                va = constp.tile([128, 3, 256], bf16, tag=f"vaug{j}",
                                 name=f"vaug{j}")
                for t in range(3):
                    nc.vector.memset(va[:, t, 0:64], 1.0)
                    nc.vector.memset(va[:, t, 128:192], 1.0)
                varing.append(va)

            PRE = 0

            def emit_v(g):
                c = ch_of[g]
                T = len(rows_s[g])
                psvg = ps_lgp.tile([128, 384], f32, tag="lg")
                for t in range(T):
                    r = rows_s[g][t]
                    off = koff_t[g][t] - ch_lo[c]
                    nc.tensor.matmul(
                        out=psvg[0:r, t * 128 : (t + 1) * 128],
                        lhsT=xkT_c[c][:, off : off + r],
                        rhs=wvT[:], start=True, stop=True,
                        skip_group_check=True,
                    )
                va = varing[g % NVR]
                nc.vector.tensor_copy(
                    out=va[:, 0:T].rearrange(
                        "p t (s f) -> p t s f", s=2)[:, :, :, 64:128],
                    in_=psvg[:, 0 : T * 128].rearrange(
                        "p (t s f) -> p t s f", t=T, s=2),
                )

            # ---- Q projection with folded Wqk ----
            for b in range(QBN):
                for h in range(4):
                    w = qb_hi[b] - qb_lo[b]
                    ps = ps_bigp.tile([128, 512], f32, tag="big")
                    nc.tensor.matmul(
                        out=ps[:, 0:w], lhsT=wqk[:, h * 128 : (h + 1) * 128],
                        rhs=xqbf_b[b][:], start=True, stop=True,
                    )
                    if h % 2 == 0:
                        nc.vector.tensor_scalar_add(
                            out=qTz_b[b][:, h, :], in0=ps[:, 0:w],
                            scalar1=bqz[:, h : h + 1],
                        )
                    else:
                        nc.scalar.activation(
                            out=qTz_b[b][:, h, :], in_=ps[:, 0:w],
                            func=AFT.Identity, bias=bqz[:, h : h + 1],
                        )

            # ---- per-slot attention, software-pipelined ----
            ex_g = {}
            pend = {}

            def emit_logits(g):
                c = ch_of[g]
                L = int(lq[g])
                b = qb_of[g]
                qo = int(qoff[g]) - qb_lo[b]
                exs = []
                for t in range(len(rows_s[g])):
                    r = rows_s[g][t]
                    off = koff_t[g][t] - ch_lo[c]
                    lg_ps = ps_lgp.tile([128, 384], f32, tag="lg")
                    nc.tensor.matmul(
                        out=lg_ps[0:r, 0 : 4 * L],
                        lhsT=xkT_c[c][:, off : off + r],
                        rhs=qTz_b[b][:, :, qo : qo + L],
                        start=True, stop=True,
                    )
                    ex = expp.tile([128, 4 * LQMAX], bf16, tag="ex")
                    kt = int(toff[g]) + t
                    nc.scalar.activation(
                        out=ex[0:r, 0 : 4 * L], in_=lg_ps[0:r, 0 : 4 * L],
                        func=AFT.Exp, bias=maskk[0:r, kt : kt + 1],
                    )
                    exs.append(ex)
                ex_g[g] = exs

            def emit_pairs(g):
                exs = ex_g.pop(g)
                L = int(lq[g])
                va = varing[g % NVR]
                pair = ps_pairp.tile([128, 4 * LQMAX], f32, tag="pair")
                T = len(rows_s[g])
                # two accumulation groups in one PSUM bank must NOT interleave
                for half in range(2):
                    for t in range(T):
                        r = rows_s[g][t]
                        nc.tensor.matmul(
                            out=pair[0:128, 2 * L * half : 2 * L * (half + 1)],
                            lhsT=va[0:r, t, 128 * half : 128 * (half + 1)],
                            rhs=exs[t][0:r, 2 * L * half : 2 * L * (half + 1)],
                            start=(t == 0), stop=(t == T - 1),
                            skip_group_check=True,
                        )
                rdb = rdenp.tile([64, 4 * LQMAX], f32, tag="rdb")
                nc.vector.reciprocal_approx_fast(
                    out=rdb[:, 0 : 4 * L], in_=pair[0:64, 0 : 4 * L])
                craw = crawp.tile([64, 4 * LQMAX], bf16, tag="craw")
                nc.vector.tensor_copy(
                    out=craw[:, 0 : 4 * L], in_=pair[64:128, 0 : 4 * L])
                qc = int(qoff[g])
                for h in range(4):
                    ro = 32 * (h % 2)
                    cs = slice(h * L, (h + 1) * L)
                    nc.gpsimd.tensor_mul(
                        out=ctxn[32 * h : 32 * (h + 1), qc : qc + L],
                        in0=craw[ro : ro + 32, cs],
                        in1=rdb[ro : ro + 32, cs],
                    )

            def emit_outproj(blk):
                sl = slice(blk * 512, (blk + 1) * 512)
                po = ps_bigp.tile([128, 512], f32, tag="big")
                nc.tensor.matmul(
                    out=po[:], lhsT=woT[:], rhs=ctxn[:, sl], start=True, stop=True
                )
                nc.vector.scalar_tensor_tensor(
                    out=ar[:, sl], in0=po[:], scalar=boc[:, 0:1],
                    in1=xqT_b[blk][:], op0=ALU.add, op1=ALU.add,
                )
                nc.vector.tensor_copy(out=arbf[:, sl], in_=ar[:, sl])

            def emit_ffn(blk):
                sl = slice(blk * 512, (blk + 1) * 512)
                pa = ps_bigp.tile([128, 512], f32, tag="big")
                nc.tensor.matmul(
                    out=pa[:], lhsT=w1T[:, 0:128], rhs=arbf[:, sl],
                    start=True, stop=True,
                )
                ra = ffnp.tile([128, 512], bf16, tag="ra")
                nc.scalar.activation(
                    out=ra[:], in_=pa[:], func=AFT.Relu, bias=b1c[:, 0:1]
                )
                pb = ps_bigp.tile([128, 512], f32, tag="big")
                nc.tensor.matmul(
                    out=pb[:], lhsT=w1T[:, 128:256], rhs=arbf[:, sl],
                    start=True, stop=True,
                )
                rb = ffnp.tile([128, 512], bf16, tag="rb")
                nc.scalar.activation(
                    out=rb[:], in_=pb[:], func=AFT.Relu, bias=b1c[:, 1:2]
                )
                p2 = ps_bigp.tile([128, 512], f32, tag="big")
                nc.tensor.matmul(
                    out=p2[:], lhsT=w2Ta[:], rhs=ra[:], start=True, stop=False,
                    skip_group_check=True,
                )
                nc.tensor.matmul(
                    out=p2[:], lhsT=w2Tb[:], rhs=rb[:], start=False, stop=True,
                    skip_group_check=True,
                )
                nc.vector.scalar_tensor_tensor(
                    out=fin[:, sl], in0=p2[:], scalar=b2c[:, 0:1],
                    in1=ar[:, sl], op0=ALU.add, op1=ALU.add,
                )
                if blk >= NQB - 2:
                    for n, i in enumerate(range(sl.start, sl.stop, 128)):
                        e = [nc.sync, nc.gpsimd, nc.scalar][n % 3]
                        e.dma_start(out=out_d[:, i : i + 128],
                                    in_=fin[:, i : i + 128])
                else:
                    nc.sync.dma_start(out=out_d[:, sl.start : sl.start + 256],
                                      in_=fin[:, sl.start : sl.start + 256])
                    nc.sync.dma_start(out=out_d[:, sl.start + 256 : sl.stop],
                                      in_=fin[:, sl.start + 256 : sl.stop])

            # outproj block blk is ready after muls of the slot covering its
            # last column
            blk_after = {}
            for blk in range(NQB):
                end = min(512 * (blk + 1), QS)
                g_ready = min(int(np.searchsorted(qoff[1:], end)), G - 1)
                blk_after.setdefault(g_ready, []).append(blk)

            LAG = 2
            for g in range(G + LAG):
                if g < G:
                    emit_v(g)
                    emit_logits(g)
                if g >= LAG:
                    emit_pairs(g - LAG)
                    for blk in blk_after.get(g - LAG, []):
                        emit_outproj(blk)
                        emit_ffn(blk)
    nc.finalize()
    return nc


_NC_CACHE = {}


def kernel(edge_index, edge_attr, incoming_edges_list, incoming_edges_batch,
           edge_batch, in_proj_w, in_proj_b, out_proj_w, out_proj_b,
           w1, b1, w2, b2):
    global LAST_RESULTS

    edge_attr = np.asarray(edge_attr, np.float32)
    edge_batch = np.asarray(edge_batch, np.int64)
    incoming_edges_list = np.asarray(incoming_edges_list, np.int64)
    incoming_edges_batch = np.asarray(incoming_edges_batch, np.int64)
    bft = ml_dtypes.bfloat16

    cnt_q = np.bincount(edge_batch, minlength=B)
    st_q = np.zeros(B + 1, np.int64)
    np.cumsum(cnt_q, out=st_q[1:])
    cnt_k = np.bincount(incoming_edges_batch, minlength=B)
    st_k = np.zeros(B + 1, np.int64)
    np.cumsum(cnt_k, out=st_k[1:])
    assert cnt_q.max() <= LQMAX and cnt_k.max() <= LKMAX and cnt_k.min() >= 1

    # sort graphs: key-tile count desc, then query count desc; deal
    # round-robin so slot s has similar sizes on all 8 cores
    tiles_g = -(-cnt_k // 128)
    order = np.lexsort((-cnt_q, -tiles_g))          # [B]
    slot_graph = order.reshape(G, NCORES)           # slot s, core c
    qmax_s = cnt_q[slot_graph].max(1)
    kmax_s = cnt_k[slot_graph].max(1)
    lq = np.minimum(-(-qmax_s // 8) * 8, LQMAX).astype(np.int64)
    rows_s = []
    for sI in range(G):
        T = int(-(-kmax_s[sI] // 128))
        last = int(kmax_s[sI]) - 128 * (T - 1)
        rows_s.append([128] * (T - 1) + [min(-(-last // 16) * 16, 128)])
    lk_s = np.array([sum(r) for r in rows_s], np.int64)

    qoff = np.r_[0, np.cumsum(lq)]
    QS = int(qoff[-1])
    NQB = -(-QS // 512)
    QSP = NQB * 512
    koff = np.r_[0, np.cumsum(lk_s)]
    KS = int(koff[-1])
    toff = np.r_[0, np.cumsum([len(r) for r in rows_s])]
    NT = int(toff[-1])

    xz = np.zeros((E + LQMAX, H), np.float32)
    xz[:E] = edge_attr
    xze = np.zeros((E + 1, H), np.float32)
    xze[:E] = edge_attr

    s = 1.0 / math.sqrt(HD)
    wq, wk, wv = in_proj_w[:H], in_proj_w[H:2 * H], in_proj_w[2 * H:]
    bq, bv = in_proj_b[:H], in_proj_b[2 * H:]
    # logits = x_q^T (s Wq_h^T Wk_h) x_k + (s Wk_h^T bq_h) . x_k  (+ col-
    # constant terms that are softmax-invariant and dropped)
    wqk = np.zeros((H, 4 * H), np.float64)
    bqz = np.zeros((H, 4), np.float64)
    for h in range(4):
        hd = slice(32 * h, 32 * (h + 1))
        wqk[:, h * H : (h + 1) * H] = s * (wq[hd].astype(np.float64).T
                                           @ wk[hd].astype(np.float64))
        bqz[:, h] = s * (wk[hd].astype(np.float64).T
                         @ bq[hd].astype(np.float64))

    shared = dict(
        wqk=wqk.astype(bft),
        bqz=bqz.astype(np.float32),
        wvT=np.ascontiguousarray(wv.T.astype(bft)),
        woT=np.ascontiguousarray(out_proj_w.T.astype(bft)),
        w1T=np.ascontiguousarray(w1.T.astype(bft)),
        w2Ta=np.ascontiguousarray(w2.T[0:128].astype(bft)),
        w2Tb=np.ascontiguousarray(w2.T[128:256].astype(bft)),
        boc=np.ascontiguousarray(
            (out_proj_b + out_proj_w @ bv)[:, None], np.float32),
        b1c=np.ascontiguousarray(b1.reshape(2, H).T, np.float32),
        b2c=np.ascontiguousarray(b2[:, None], np.float32),
    )

    in_maps = []
    for c in range(NCORES):
        gs = slot_graph[:, c]
        xq_c = np.zeros((QSP, H), np.float32)
        xk_c = np.zeros((KS, H), np.float32)
        mk = np.full((128, NT), MASK_VAL, np.float32)
        for sI in range(G):
            g = gs[sI]
            L = int(lq[sI])
            xq_c[qoff[sI] : qoff[sI] + L] = xz[st_q[g] : st_q[g] + L]
            n_k = int(cnt_k[g])
            idx = incoming_edges_list[st_k[g] : st_k[g] + n_k]
            xk_c[koff[sI] : koff[sI] + n_k] = xze[idx]
            pos = 0
            for t, r in enumerate(rows_s[sI]):
                v = np.arange(r) + pos
                mk[:r, toff[sI] + t] = np.where(v < n_k, 0.0, MASK_VAL)
                pos += r
        in_maps.append(dict(
            shared,
            xqbf=np.ascontiguousarray(xq_c.T.astype(bft)),
            xkT=np.ascontiguousarray(xk_c.T.astype(bft)),
            maskk=np.ascontiguousarray(mk),
        ))

    key = (tuple(int(x) for x in lq), tuple(tuple(r) for r in rows_s))
    if key not in _NC_CACHE:
        _NC_CACHE.clear()
        _NC_CACHE[key] = _build_program(lq, rows_s)
    res = run_bass_kernel_spmd(
        _NC_CACHE[key], in_maps, core_ids=list(range(NCORES)),
        trace=TRACE, **TRACE_KW,
    )
    LAST_RESULTS = res

    # scatter dense output back to edge rows
    slot_of = np.empty(B, np.int64)
    core_of = np.empty(B, np.int64)
    for sI in range(G):
        for c in range(NCORES):
            slot_of[slot_graph[sI, c]] = sI
            core_of[slot_graph[sI, c]] = c
    eb = edge_batch
    pos = np.arange(E) - st_q[eb]
    col = qoff[slot_of[eb]] + pos
    out_full = np.empty((E, H), np.float32)
    for c in range(NCORES):
        sel = core_of[eb] == c
        out_full[sel] = res.results[c]["out"][:, col[sel]].T.astype(np.float32)
    return out_full


# revision 35
# speedup vs baseline: 1.0434x; 1.0203x over previous
"""Trainium2 Bass kernel for nn_MessageAggregationAttention.

Shards B=256 graphs across 8 NeuronCores (32 graph-slots each). The host does
all data layout (gather of incoming-message rows, per-slot padding,
feature-major transposes) and weight-only algebra:

  - graphs are sorted by key-tile count then query count and dealt
    round-robin to the 8 cores, so slot s has the same static padded sizes
    (lq[s], key tile rows) on every core; the program is compiled for that
    size signature (ragged per-slot matmul shapes),
  - logits are computed as x_q^T (s Wq_h^T Wk_h) x_k: the per-head weight
    product Wqk_h is precomputed on the host, so the device needs NO separate
    K projection; the q-side bias s Wk_h^T bq_h folds into the q bias, the
    k-side bias only shifts whole softmax columns and drops exactly,
  - per (slot, key-tile): logits matmul (lhsT = raw xkT) -> Exp (key-padding
    mask as activation bias) -> two "pair" matmuls with an ones-augmented V
    (lhsT = [ones64 | v_h0 v_h1]) accumulating ctx for two heads AND the
    softmax denominator (partitions 0:64) in one pass,
  - normalize: reciprocal + one pair-flatten copy (vector) + SBUF-only
    per-head multiplies (gpsimd; gpsimd cannot touch PSUM),
  - batched out-projection and FFN with fused bias+residual adds
    (scalar_tensor_tensor), bf16 residual/output (host upcasts).

Hardware notes baked into the structure: PSUM accumulation groups sharing a
bank must not interleave; DVE ops may read at most one PSUM operand and two
SBUF inputs must share a base partition (mixed-space operands may differ);
custom-DVE reciprocal requires base partition 0 and f32; each dma_start
rides one DMA queue (~22 GB/s) and costs ~0.7us of issuing-engine sequencer
time, so large tensors are split and issuance is spread across sync/gpsimd/
scalar DGEs at startup and for the output drain.
"""

import math

import ml_dtypes
import numpy as np

import concourse.mybir as mybir
from concourse import bacc
from concourse.bass_utils import run_bass_kernel_spmd
from concourse.tile import TileContext

B, E, M, H, NH = 256, 16384, 65536, 128, 4
HD = H // NH               # 32
NCORES = 8
G = B // NCORES            # 32 graph slots per core
LQMAX, LKMAX = 96, 320
MASK_VAL = -100.0

f32 = mybir.dt.float32
bf16 = mybir.dt.bfloat16

AFT = mybir.ActivationFunctionType
ALU = mybir.AluOpType

LAST_RESULTS = None
TRACE = False
TRACE_KW = {}


def _build_program(lq, rows_s):
    """lq[s]: padded query count of slot s; rows_s[s]: key-tile row counts
    (e.g. [128, 128, 48])."""
    nc = bacc.Bacc("TRN2")

    qoff = np.r_[0, np.cumsum(lq)]
    QS = int(qoff[-1])
    NQB = -(-QS // 512)
    QSP = NQB * 512
    koff_t = []                        # (slot, tile) -> key-slab col offset
    ko = 0
    for s in range(G):
        offs = []
        for r in rows_s[s]:
            offs.append(ko)
            ko += r
        koff_t.append(offs)
    KS = ko
    toff = np.r_[0, np.cumsum([len(r) for r in rows_s])]  # mask col offsets
    NT = int(toff[-1])

    xqbf_d = nc.dram_tensor("xqbf", [H, QSP], bf16, kind="ExternalInput")
    xkT_d = nc.dram_tensor("xkT", [H, KS], bf16, kind="ExternalInput")
    maskk_d = nc.dram_tensor("maskk", [128, NT], f32, kind="ExternalInput")
    wqk_d = nc.dram_tensor("wqk", [H, 4 * H], bf16, kind="ExternalInput")
    bqz_d = nc.dram_tensor("bqz", [H, 4], f32, kind="ExternalInput")
    wvT_d = nc.dram_tensor("wvT", [H, H], bf16, kind="ExternalInput")
    woT_d = nc.dram_tensor("woT", [H, H], bf16, kind="ExternalInput")
    w1T_d = nc.dram_tensor("w1T", [H, 2 * H], bf16, kind="ExternalInput")
    w2Ta_d = nc.dram_tensor("w2Ta", [128, H], bf16, kind="ExternalInput")
    w2Tb_d = nc.dram_tensor("w2Tb", [128, H], bf16, kind="ExternalInput")
    boc_d = nc.dram_tensor("boc", [H, 1], f32, kind="ExternalInput")
    b1c_d = nc.dram_tensor("b1c", [H, 2], f32, kind="ExternalInput")
    b2c_d = nc.dram_tensor("b2c", [H, 1], f32, kind="ExternalInput")

    out_d = nc.dram_tensor("out", [H, QSP], bf16, kind="ExternalOutput")

    NVR = 4
    with TileContext(nc) as tc:
        with (
            tc.tile_pool(name="const", bufs=1) as constp,
            tc.tile_pool(name="exp", bufs=12) as expp,
            tc.tile_pool(name="rden", bufs=3) as rdenp,
            tc.tile_pool(name="craw", bufs=4) as crawp,
            tc.tile_pool(name="ffn", bufs=3) as ffnp,
            tc.tile_pool(name="ps_big", bufs=2, space="PSUM") as ps_bigp,
            tc.tile_pool(name="ps_lg", bufs=4, space="PSUM") as ps_lgp,
            tc.tile_pool(name="ps_pair", bufs=2, space="PSUM") as ps_pairp,
        ):
            dma_engs = [nc.sync, nc.gpsimd, nc.scalar]
            dma_rr = [0]

            def _dma(out, in_):
                # parallel DGE issuance only for the first (startup-critical)
                # loads; everything later stays on the idle sync queue
                e = dma_engs[dma_rr[0] % 3] if dma_rr[0] < 6 else nc.sync
                e.dma_start(out=out, in_=in_)
                dma_rr[0] += 1

            def _load(shape, dram, dt=f32, nsplit=1):
                t = constp.tile(shape, dt, tag=dram.name, name=dram.name + "_sb")
                w = shape[1]
                step = -(-w // nsplit)
                for i in range(0, w, step):
                    j = min(i + step, w)
                    _dma(t[:, i:j], dram[:, i:j])
                return t

            # DMA issue order matters for the startup critical path:
            # V-proj (wvT + xkT c0) starts first, then the Q side.
            wqk = _load([H, 4 * H], wqk_d, bf16, nsplit=3)
            bqz = _load([H, 4], bqz_d)
            # xqbf blocks gate qproj; 4-slot blocks also gate logits
            QBN = G // 4
            qb_of = [s // 4 for s in range(G)]
            qb_lo = [int(qoff[4 * b]) for b in range(QBN)]
            qb_hi = [int(qoff[4 * b + 4]) for b in range(QBN)]
            xqbf_b = []
            for b in range(QBN):
                w = qb_hi[b] - qb_lo[b]
                t = constp.tile([128, w], bf16, tag=f"xqbf{b}", name=f"xqbf{b}")
                _dma(t[:], xqbf_d[:, qb_lo[b] : qb_hi[b]])
                xqbf_b.append(t)
            wvT = _load([H, H], wvT_d, bf16)
            maskk = _load([128, NT], maskk_d)
            # xkT chunks of 8 slots
            NCH = 4
            ch_of = [s // 8 for s in range(G)]
            ch_lo = [koff_t[8 * c][0] for c in range(NCH)]
            ch_hi = [koff_t[8 * c + 7][-1] + rows_s[8 * c + 7][-1]
                     for c in range(NCH)]
            xkT_c = []
            for c in range(NCH):
                w = ch_hi[c] - ch_lo[c]
                t = constp.tile([128, w], bf16, tag=f"xkT{c}", name=f"xkT{c}")
                step = -(-w // 2)
                for n, i in enumerate(range(0, w, step)):
                    j = min(i + step, w)
                    _dma(t[:, i:j], xkT_d[:, ch_lo[c] + i : ch_lo[c] + j])
                xkT_c.append(t)
            xqT_b = []
            for b5 in range(NQB):
                t = constp.tile([128, 512], bf16, tag=f"xqTb{b5}",
                                name=f"xqTb{b5}")
                _dma(t[:], xqbf_d[:, b5 * 512 : (b5 + 1) * 512])
                xqT_b.append(t)
            woT = _load([H, H], woT_d, bf16)
            w1T = _load([H, 2 * H], w1T_d, bf16)
            w2Ta = _load([128, H], w2Ta_d, bf16)
            w2Tb = _load([128, H], w2Tb_d, bf16)
            boc = _load([H, 1], boc_d)
            b1c = _load([H, 2], b1c_d)
            b2c = _load([H, 1], b2c_d)

            qTz_b = [constp.tile([128, 4, qb_hi[b] - qb_lo[b]], bf16,
                                 tag=f"qTz{b}", name=f"qTz{b}")
                     for b in range(QBN)]
            ctxn = constp.tile([128, QSP], bf16, tag="ctxn", name="ctxn")
            ar = constp.tile([128, QSP], f32, tag="ar", name="ar")
            arbf = constp.tile([128, QSP], bf16, tag="arbf", name="arbf")
            fin = constp.tile([128, QSP], bf16, tag="fin", name="fin")
            if QSP > QS:
                nc.vector.memset(ctxn[:, QS:QSP], 0.0)

            # v_aug ring: per slot [128, T, 256]:
            # [ones64 | v_h0 v_h1 | ones64 | v_h2 v_h3]
            varing = []
            for j in range(NVR):
                va = constp.tile([128, 3, 256], bf16, tag=f"vaug{j}",
                                 name=f"vaug{j}")
                for t in range(3):
                    nc.vector.memset(va[:, t, 0:64], 1.0)
                    nc.vector.memset(va[:, t, 128:192], 1.0)
                varing.append(va)

            PRE = 0

            def emit_v(g):
                c = ch_of[g]
                T = len(rows_s[g])
                psvg = ps_lgp.tile([128, 384], f32, tag="lg")
                for t in range(T):
                    r = rows_s[g][t]
                    off = koff_t[g][t] - ch_lo[c]
                    nc.tensor.matmul(
                        out=psvg[0:r, t * 128 : (t + 1) * 128],
                        lhsT=xkT_c[c][:, off : off + r],
                        rhs=wvT[:], start=True, stop=True,
                        skip_group_check=True,
                    )
                va = varing[g % NVR]
                nc.vector.tensor_copy(
                    out=va[:, 0:T].rearrange(
                        "p t (s f) -> p t s f", s=2)[:, :, :, 64:128],
                    in_=psvg[:, 0 : T * 128].rearrange(
                        "p (t s f) -> p t s f", t=T, s=2),
                )

            # ---- Q projection with folded Wqk ----
            for b in range(QBN):
                for h in range(4):
                    w = qb_hi[b] - qb_lo[b]
                    ps = ps_bigp.tile([128, 512], f32, tag="big")
                    nc.tensor.matmul(
                        out=ps[:, 0:w], lhsT=wqk[:, h * 128 : (h + 1) * 128],
                        rhs=xqbf_b[b][:], start=True, stop=True,
                    )
                    if h % 2 == 0:
                        nc.vector.tensor_scalar_add(
                            out=qTz_b[b][:, h, :], in0=ps[:, 0:w],
                            scalar1=bqz[:, h : h + 1],
                        )
                    else:
                        nc.scalar.activation(
                            out=qTz_b[b][:, h, :], in_=ps[:, 0:w],
                            func=AFT.Identity, bias=bqz[:, h : h + 1],
                        )

            # ---- per-slot attention, software-pipelined ----
            ex_g = {}
            pend = {}

            def emit_logits(g):
                c = ch_of[g]
                L = int(lq[g])
                b = qb_of[g]
                qo = int(qoff[g]) - qb_lo[b]
                exs = []
                for t in range(len(rows_s[g])):
                    r = rows_s[g][t]
                    off = koff_t[g][t] - ch_lo[c]
                    lg_ps = ps_lgp.tile([128, 384], f32, tag="lg")
                    nc.tensor.matmul(
                        out=lg_ps[0:r, 0 : 4 * L],
                        lhsT=xkT_c[c][:, off : off + r],
                        rhs=qTz_b[b][:, :, qo : qo + L],
                        start=True, stop=True,
                    )
                    ex = expp.tile([128, 4 * LQMAX], bf16, tag="ex")
                    kt = int(toff[g]) + t
                    nc.scalar.activation(
                        out=ex[0:r, 0 : 4 * L], in_=lg_ps[0:r, 0 : 4 * L],
                        func=AFT.Exp, bias=maskk[0:r, kt : kt + 1],
                    )
                    exs.append(ex)
                ex_g[g] = exs

            def emit_pairs(g):
                exs = ex_g.pop(g)
                L = int(lq[g])
                va = varing[g % NVR]
                pair = ps_pairp.tile([128, 4 * LQMAX], f32, tag="pair")
                T = len(rows_s[g])
                # two accumulation groups in one PSUM bank must NOT interleave
                for half in range(2):
                    for t in range(T):
                        r = rows_s[g][t]
                        nc.tensor.matmul(
                            out=pair[0:128, 2 * L * half : 2 * L * (half + 1)],
                            lhsT=va[0:r, t, 128 * half : 128 * (half + 1)],
                            rhs=exs[t][0:r, 2 * L * half : 2 * L * (half + 1)],
                            start=(t == 0), stop=(t == T - 1),
                            skip_group_check=True,
                        )
                rdb = rdenp.tile([64, 4 * LQMAX], f32, tag="rdb")
                nc.vector.reciprocal_approx_fast(
                    out=rdb[:, 0 : 4 * L], in_=pair[0:64, 0 : 4 * L])
                craw = crawp.tile([64, 4 * LQMAX], bf16, tag="craw")
                nc.vector.tensor_copy(
                    out=craw[:, 0 : 4 * L], in_=pair[64:128, 0 : 4 * L])
                qc = int(qoff[g])
                for h in range(4):
                    ro = 32 * (h % 2)
                    cs = slice(h * L, (h + 1) * L)
                    nc.gpsimd.tensor_mul(
                        out=ctxn[32 * h : 32 * (h + 1), qc : qc + L],
                        in0=craw[ro : ro + 32, cs],
                        in1=rdb[ro : ro + 32, cs],
                    )

            def emit_outproj(blk):
                sl = slice(blk * 512, (blk + 1) * 512)
                po = ps_bigp.tile([128, 512], f32, tag="big")
                nc.tensor.matmul(
                    out=po[:], lhsT=woT[:], rhs=ctxn[:, sl], start=True, stop=True
                )
                nc.vector.scalar_tensor_tensor(
                    out=ar[:, sl], in0=po[:], scalar=boc[:, 0:1],
                    in1=xqT_b[blk][:], op0=ALU.add, op1=ALU.add,
                )
                nc.vector.tensor_copy(out=arbf[:, sl], in_=ar[:, sl])

            def emit_ffn(blk):
                sl = slice(blk * 512, (blk + 1) * 512)
                pa = ps_bigp.tile([128, 512], f32, tag="big")
                nc.tensor.matmul(
                    out=pa[:], lhsT=w1T[:, 0:128], rhs=arbf[:, sl],
                    start=True, stop=True,
                )
                ra = ffnp.tile([128, 512], bf16, tag="ra")
                nc.scalar.activation(
                    out=ra[:], in_=pa[:], func=AFT.Relu, bias=b1c[:, 0:1]
                )
                pb = ps_bigp.tile([128, 512], f32, tag="big")
                nc.tensor.matmul(
                    out=pb[:], lhsT=w1T[:, 128:256], rhs=arbf[:, sl],
                    start=True, stop=True,
                )
                rb = ffnp.tile([128, 512], bf16, tag="rb")
                nc.scalar.activation(
                    out=rb[:], in_=pb[:], func=AFT.Relu, bias=b1c[:, 1:2]
                )
                p2 = ps_bigp.tile([128, 512], f32, tag="big")
                nc.tensor.matmul(
                    out=p2[:], lhsT=w2Ta[:], rhs=ra[:], start=True, stop=False,
                    skip_group_check=True,
                )
                nc.tensor.matmul(
                    out=p2[:], lhsT=w2Tb[:], rhs=rb[:], start=False, stop=True,
                    skip_group_check=True,
                )
                nc.vector.scalar_tensor_tensor(
                    out=fin[:, sl], in0=p2[:], scalar=b2c[:, 0:1],
                    in1=ar[:, sl], op0=ALU.add, op1=ALU.add,
                )
                if blk >= NQB - 2:
                    for n, i in enumerate(range(sl.start, sl.stop, 128)):
                        e = [nc.sync, nc.gpsimd, nc.scalar][n % 3]
                        e.dma_start(out=out_d[:, i : i + 128],
                                    in_=fin[:, i : i + 128])
                else:
                    nc.sync.dma_start(out=out_d[:, sl.start : sl.start + 256],
                                      in_=fin[:, sl.start : sl.start + 256])
                    nc.sync.dma_start(out=out_d[:, sl.start + 256 : sl.stop],
                                      in_=fin[:, sl.start + 256 : sl.stop])

            # outproj block blk is ready after muls of the slot covering its
            # last column
            blk_after = {}
            for blk in range(NQB):
                end = min(512 * (blk + 1), QS)
                g_ready = min(int(np.searchsorted(qoff[1:], end)), G - 1)
                blk_after.setdefault(g_ready, []).append(blk)

            LAG = 2
            for g in range(G + LAG):
                if g < G:
                    emit_v(g)
                    emit_logits(g)
                if g >= LAG:
                    emit_pairs(g - LAG)
                    for blk in blk_after.get(g - LAG, []):
                        emit_outproj(blk)
                        emit_ffn(blk)
    nc.finalize()
    return nc


_NC_CACHE = {}


def kernel(edge_index, edge_attr, incoming_edges_list, incoming_edges_batch,
           edge_batch, in_proj_w, in_proj_b, out_proj_w, out_proj_b,
           w1, b1, w2, b2):
    global LAST_RESULTS

    edge_attr = np.asarray(edge_attr, np.float32)
    edge_batch = np.asarray(edge_batch, np.int64)
    incoming_edges_list = np.asarray(incoming_edges_list, np.int64)
    incoming_edges_batch = np.asarray(incoming_edges_batch, np.int64)
    bft = ml_dtypes.bfloat16

    cnt_q = np.bincount(edge_batch, minlength=B)
    st_q = np.zeros(B + 1, np.int64)
    np.cumsum(cnt_q, out=st_q[1:])
    cnt_k = np.bincount(incoming_edges_batch, minlength=B)
    st_k = np.zeros(B + 1, np.int64)
    np.cumsum(cnt_k, out=st_k[1:])
    assert cnt_q.max() <= LQMAX and cnt_k.max() <= LKMAX and cnt_k.min() >= 1

    # sort graphs: key-tile count desc, then query count desc; deal
    # round-robin so slot s has similar sizes on all 8 cores
    tiles_g = -(-cnt_k // 128)
    order = np.lexsort((-cnt_q, -tiles_g))          # [B]
    slot_graph = order.reshape(G, NCORES)           # slot s, core c
    qmax_s = cnt_q[slot_graph].max(1)
    kmax_s = cnt_k[slot_graph].max(1)
    lq = np.minimum(-(-qmax_s // 8) * 8, LQMAX).astype(np.int64)
    rows_s = []
    for sI in range(G):
        T = int(-(-kmax_s[sI] // 128))
        last = int(kmax_s[sI]) - 128 * (T - 1)
        rows_s.append([128] * (T - 1) + [min(-(-last // 16) * 16, 128)])
    lk_s = np.array([sum(r) for r in rows_s], np.int64)

    qoff = np.r_[0, np.cumsum(lq)]
    QS = int(qoff[-1])
    NQB = -(-QS // 512)
    QSP = NQB * 512
    koff = np.r_[0, np.cumsum(lk_s)]
    KS = int(koff[-1])
    toff = np.r_[0, np.cumsum([len(r) for r in rows_s])]
    NT = int(toff[-1])

    xz = np.zeros((E + LQMAX, H), np.float32)
    xz[:E] = edge_attr
    xze = np.zeros((E + 1, H), np.float32)
    xze[:E] = edge_attr

    s = 1.0 / math.sqrt(HD)
    wq, wk, wv = in_proj_w[:H], in_proj_w[H:2 * H], in_proj_w[2 * H:]
    bq, bv = in_proj_b[:H], in_proj_b[2 * H:]
    # logits = x_q^T (s Wq_h^T Wk_h) x_k + (s Wk_h^T bq_h) . x_k  (+ col-
    # constant terms that are softmax-invariant and dropped)
    wqk = np.zeros((H, 4 * H), np.float64)
    bqz = np.zeros((H, 4), np.float64)
    for h in range(4):
        hd = slice(32 * h, 32 * (h + 1))
        wqk[:, h * H : (h + 1) * H] = s * (wq[hd].astype(np.float64).T
                                           @ wk[hd].astype(np.float64))
        bqz[:, h] = s * (wk[hd].astype(np.float64).T
                         @ bq[hd].astype(np.float64))

    shared = dict(
        wqk=wqk.astype(bft),
        bqz=bqz.astype(np.float32),
        wvT=np.ascontiguousarray(wv.T.astype(bft)),
        woT=np.ascontiguousarray(out_proj_w.T.astype(bft)),
        w1T=np.ascontiguousarray(w1.T.astype(bft)),
        w2Ta=np.ascontiguousarray(w2.T[0:128].astype(bft)),
        w2Tb=np.ascontiguousarray(w2.T[128:256].astype(bft)),
        boc=np.ascontiguousarray(
            (out_proj_b + out_proj_w @ bv)[:, None], np.float32),
        b1c=np.ascontiguousarray(b1.reshape(2, H).T, np.float32),
        b2c=np.ascontiguousarray(b2[:, None], np.float32),
    )

    in_maps = []
    for c in range(NCORES):
        gs = slot_graph[:, c]
        xq_c = np.zeros((QSP, H), np.float32)
        xk_c = np.zeros((KS, H), np.float32)
        mk = np.full((128, NT), MASK_VAL, np.float32)
        for sI in range(G):
            g = gs[sI]
            L = int(lq[sI])
            xq_c[qoff[sI] : qoff[sI] + L] = xz[st_q[g] : st_q[g] + L]
            n_k = int(cnt_k[g])
            idx = incoming_edges_list[st_k[g] : st_k[g] + n_k]
            xk_c[koff[sI] : koff[sI] + n_k] = xze[idx]
            pos = 0
            for t, r in enumerate(rows_s[sI]):
                v = np.arange(r) + pos
                mk[:r, toff[sI] + t] = np.where(v < n_k, 0.0, MASK_VAL)
                pos += r
        in_maps.append(dict(
            shared,
            xqbf=np.ascontiguousarray(xq_c.T.astype(bft)),
            xkT=np.ascontiguousarray(xk_c.T.astype(bft)),
            maskk=np.ascontiguousarray(mk),
        ))

    key = (tuple(int(x) for x in lq), tuple(tuple(r) for r in rows_s))
    if key not in _NC_CACHE:
        _NC_CACHE.clear()
        _NC_CACHE[key] = _build_program(lq, rows_s)
    res = run_bass_kernel_spmd(
        _NC_CACHE[key], in_maps, core_ids=list(range(NCORES)),
        trace=TRACE, **TRACE_KW,
    )
    LAST_RESULTS = res

    # scatter dense output back to edge rows
    slot_of = np.empty(B, np.int64)
    core_of = np.empty(B, np.int64)
    for sI in range(G):
        for c in range(NCORES):
            slot_of[slot_graph[sI, c]] = sI
            core_of[slot_graph[sI, c]] = c
    eb = edge_batch
    pos = np.arange(E) - st_q[eb]
    col = qoff[slot_of[eb]] + pos
    out_full = np.empty((E, H), np.float32)
    for c in range(NCORES):
        sel = core_of[eb] == c
        out_full[sel] = res.results[c]["out"][:, col[sel]].T.astype(np.float32)
    return out_full
